# revision 3
# baseline (speedup 1.0000x reference)
"""Trainium2 Bass kernel for nn_CustomLoss_45449343926664 (retrieval_knn).

loss = mse(mean(c1), mean(c2))
     + mean_i min_j ||c1_i - c2_j||^2
     + mean_k relu(0.1 - var(c1)_k)

Device computes the dominant term: per-row max_j(2<c1_i,c2_j> - |c2_j|^2)
(min-distance via d2 = |c1_i|^2 - that max). The tiny O(N*D) stats
(means / variances / |c1_i|^2) are host-side in fp64, fused into the
final scalar in _finish.

Sharding (8 cores = 4 i-groups x 2 j-halves): core c owns c1 rows
[2048*(c%4), 2048*(c%4+1)) and c2 rows [4096*(c//4), 4096*(c//4+1)).
This doubles the drain free-dim (2048 i per instruction) vs the natural
1024, halving per-instruction overhead on the drain engines.

Per core: 32 j-tiles of 128. Cross matmuls in "j-on-partitions"
orientation (c2bT tile stationary, c1bT moving, bf16, c1 pre-scaled by
2): psum tile [128 j, 2048 i] = 4 banks, built by 4 FD=512 matmuls.
Drain is split across DVE and ACT (the only engines with PSUM read
ports), balanced so both run ~equally busy:

  - 8 j-tiles: DVE fused scalar_tensor_tensor drain
        zD' = max(psum + bias_j, zD)        (1 pass, ping-pong accum)
  - 24 j-tiles: ACT activation(Identity, bias_j) -> bf16 z halves; each
    PAIR of z tiles is folded by one DVE bf16 tensor_tensor max (2x
    mode, [128, 4096]) into the two independent halves of the zA
    ping-pong accumulator.

Tail: zfin = max(zD, zA halves); 16 PE transposes + one 3D reduce_max
give gmax[p, b] = max_j(...) for query i = 128*b + p (+ 2048*(c%4)).
Host combines the two j-halves, adds |c1_i|^2, and averages.
"""
import os
import sys

import numpy as np
import ml_dtypes

if os.path.isdir("/opt/trn_rl_repo") and "/opt/trn_rl_repo" not in sys.path:
    sys.path.insert(0, "/opt/trn_rl_repo")

from contextlib import ExitStack

import concourse.bass as bass
import concourse.tile as tile
from concourse import bacc, mybir
from concourse.bass_utils import run_bass_kernel_spmd
from concourse.masks import make_identity

F32 = mybir.dt.float32
BF16 = mybir.dt.bfloat16
BF16_NP = ml_dtypes.bfloat16
NEG_BIG = -3.0e38

N_CORES = 8
N1 = 8192            # cluster1 rows (total)
N2 = 8192            # cluster2 rows
D = 128              # feature dim = partition count
P = 128
I_GROUPS = 4
J_HALVES = 2
NI = N1 // I_GROUPS  # 2048 c1 rows per core
NJ = N2 // J_HALVES  # 4096 c2 rows per core
NJT = NJ // P        # 32 j-tiles of 128
MTI = NI // P        # 16 i-blocks of 128 (for the transpose tail)
MM_FD = 512          # matmul free-dim (one PSUM bank)
MM_PER_TILE = NI // MM_FD   # 4 matmuls per j-tile

# j-tiles drained directly by DVE (8 of 32); the other 24 go to ACT.
DVE_TILES = frozenset(range(0, NJT, 4))
MIN_VARIANCE = 0.1

_cached = {}


def _build_program():
    """Build + compile the single-core SPMD program (same for all cores)."""
    nc = bacc.Bacc(
        "TRN2",
        target_bir_lowering=False,
        debug=False,
        enable_asserts=False,
        num_devices=N_CORES,
    )

    d_c1bT = nc.dram_tensor("c1bT", [D, NI], BF16, kind="ExternalInput").ap()
    d_c2bT = nc.dram_tensor("c2bT", [D, NJ], BF16, kind="ExternalInput").ap()
    d_sq2neg = nc.dram_tensor("sq2neg", [P, NJT], F32, kind="ExternalInput").ap()

    d_gmax = nc.dram_tensor("gmax", [P, MTI], F32, kind="ExternalOutput").ap()

    with tile.TileContext(nc) as tc, ExitStack() as ctx:
        const = ctx.enter_context(tc.tile_pool(name="const", bufs=1))
        zring = ctx.enter_context(tc.tile_pool(name="zring", bufs=3))
        psum = ctx.enter_context(tc.tile_pool(name="psum", bufs=2, space="PSUM"))

        t_c1bT = const.tile([P, NI], BF16)
        t_c2bT = const.tile([P, NJ], BF16)
        t_sq2neg = const.tile([P, NJT], F32)
        t_zA = [const.tile([P, 2, NI], BF16, name=f"zA{i}") for i in range(2)]
        t_zD = [const.tile([P, NI], BF16, name=f"zD{i}") for i in range(2)]
        t_zfin = const.tile([P, NI], BF16)
        t_gmax = const.tile([P, MTI], F32)
        t_ident = const.tile([P, P], BF16)
        t_dummy = const.tile([P, 1], F32)

        # warm the ACT table set immediately (table load ~1.3us, needed
        # before the first real drain); depends only on a fast memset
        nc.vector.memset(t_dummy[:], 1.0)
        nc.scalar.activation(t_dummy[:], t_dummy[:],
                             mybir.ActivationFunctionType.Identity, bias=0.0)

        # accumulator init + identity (for the PE transposes) off the
        # critical path on gpsimd/vector
        make_identity(nc, t_ident[:])
        nc.gpsimd.memset(t_zA[0][:], NEG_BIG)
        nc.gpsimd.memset(t_zD[0][:], NEG_BIG)

        # ---- input DMAs in priority order ----
        # first j-tiles' stationaries + the full moving tensor come first
        nc.sync.dma_start(t_sq2neg[:], d_sq2neg)
        nc.sync.dma_start(t_c2bT[:, 0 : 4 * P], d_c2bT[:, 0 : 4 * P])
        nc.scalar.dma_start(t_c1bT[:, 0 : NI // 2], d_c1bT[:, 0 : NI // 2])
        nc.sync.dma_start(t_c1bT[:, NI // 2 :], d_c1bT[:, NI // 2 :])
        for ci in range(4):
            lo, hi = 4 * P + ci * 896, min(4 * P + (ci + 1) * 896, NJ)
            eng = nc.scalar if ci % 2 == 0 else nc.sync
            eng.dma_start(t_c2bT[:, lo:hi], d_c2bT[:, lo:hi])

        # ---- cross matmuls (j on partitions) + dual-engine drain ----
        nd = na = 0
        zhalf = 0
        zt = None
        for t in range(NJT):
            pt = psum.tile([P, NI], F32, tag="pcross", name="pcross")
            lhsT = t_c2bT[:, t * P : (t + 1) * P]
            for c in range(MM_PER_TILE):
                nc.tensor.matmul(
                    pt[:, c * MM_FD : (c + 1) * MM_FD],
                    lhsT,
                    t_c1bT[:, c * MM_FD : (c + 1) * MM_FD],
                    start=True,
                    stop=True,
                )
            bias = t_sq2neg[:, t : t + 1]
            if t in DVE_TILES:
                nc.vector.scalar_tensor_tensor(
                    out=t_zD[(nd + 1) % 2][:],
                    in0=pt[:],
                    scalar=bias,
                    in1=t_zD[nd % 2][:],
                    op0=mybir.AluOpType.add,
                    op1=mybir.AluOpType.max,
                )
                nd += 1
            else:
                if zhalf == 0:
                    zt = zring.tile([P, 2, NI], BF16, name="zt")
                nc.scalar.activation(
                    zt[:, zhalf], pt[:], mybir.ActivationFunctionType.Identity,
                    bias=bias, scale=1.0,
                )
                if zhalf == 1:
                    # one bf16 2x tensor_max folds both tiles of the pair
                    # into the two independent halves of the zA accumulator
                    nc.vector.tensor_max(t_zA[(na + 1) % 2][:],
                                         t_zA[na % 2][:], zt[:])
                    na += 1
                zhalf ^= 1

        # ---- tail: combine partial maxes, partition-reduce via PE transpose
        nc.vector.tensor_max(t_zfin[:], t_zD[nd % 2][:], t_zA[na % 2][:, 0])
        nc.vector.tensor_max(t_zfin[:], t_zfin[:], t_zA[na % 2][:, 1])
        ptr = psum.tile([P, MTI, P], BF16, tag="pcross", name="ptr")
        for c in range(MTI):
            nc.tensor.transpose(ptr[:, c], t_zfin[:, c * P : (c + 1) * P],
                                t_ident[:])
        nc.vector.tensor_reduce(t_gmax[:], ptr[:], axis=mybir.AxisListType.X,
                                op=mybir.AluOpType.max)
        nc.sync.dma_start(d_gmax, t_gmax[:])

    nc.compile()
    return nc


def _prep_inputs(cluster1: np.ndarray, cluster2: np.ndarray):
    """Host-side sharding + operand layout prep."""
    c2b = cluster2.astype(BF16_NP)
    c2bT = np.ascontiguousarray(c2b.T)                       # [128, 8192] bf16
    sq2 = (c2b.astype(np.float32) ** 2).sum(axis=1)          # [8192] fp32
    sq2neg_h = []
    c2bT_h = []
    for h in range(J_HALVES):
        s = (-sq2[h * NJ : (h + 1) * NJ]).reshape(NJT, P).T
        sq2neg_h.append(np.ascontiguousarray(s).astype(np.float32))
        c2bT_h.append(np.ascontiguousarray(c2bT[:, h * NJ : (h + 1) * NJ]))

    c1bT_g = []
    for g in range(I_GROUPS):
        c1s = cluster1[g * NI : (g + 1) * NI]
        c1bT_g.append(np.ascontiguousarray((2.0 * c1s).astype(BF16_NP).T))

    in_maps = []
    for c in range(N_CORES):
        g, h = c % I_GROUPS, c // I_GROUPS
        in_maps.append({
            "c1bT": c1bT_g[g],
            "c2bT": c2bT_h[h],
            "sq2neg": sq2neg_h[h],
        })
    return in_maps


def _finish(results, cluster1, cluster2) -> np.float32:
    """Combine per-core partials + host-side O(N*D) stats (fp64)."""
    c1 = np.asarray(cluster1, np.float64)
    c2 = np.asarray(cluster2, np.float64)

    # distance term: d2_i = |c1_i|^2 - max_j(2<c1,c2> - |c2_j|^2)
    # (cross/bias computed on device from bf16-rounded operands)
    sq1 = (c1 * c1).sum(axis=1)  # [8192]
    dist_sum = 0.0
    for g in range(I_GROUPS):
        gm0 = np.asarray(results[g]["gmax"], np.float64)            # [128, 16]
        gm1 = np.asarray(results[g + I_GROUPS]["gmax"], np.float64)
        gm = np.maximum(gm0, gm1)                  # [p, b]; i_local = 128*b + p
        gmax_rows = gm.T.reshape(NI)               # [2048] in i_local order
        dist_sum += (sq1[g * NI : (g + 1) * NI] - gmax_rows).sum()
    dist = dist_sum / N1

    m1 = c1.mean(axis=0)
    m2 = c2.mean(axis=0)
    mean_loss = ((m1 - m2) ** 2).mean()
    var = (c1 * c1).mean(axis=0) - m1 ** 2
    disp = np.maximum(MIN_VARIANCE - var, 0.0).mean()
    return np.float32(mean_loss + dist + disp)


def _run(inputs, trace=False, **kwargs):
    """Run on the 8 NeuronCores. Returns (loss_scalar, BassKernelResults)."""
    if "nc" not in _cached:
        _cached["nc"] = _build_program()
    nc = _cached["nc"]
    c1 = np.asarray(inputs["cluster1"], np.float32)
    c2 = np.asarray(inputs["cluster2"], np.float32)
    in_maps = _prep_inputs(c1, c2)
    res = run_bass_kernel_spmd(nc, in_maps, list(range(N_CORES)), trace=trace,
                               **kwargs)
    loss = _finish(res.results, c1, c2)
    return loss, res


def kernel(cluster1: np.ndarray, cluster2: np.ndarray) -> np.ndarray:
    loss, _ = _run({"cluster1": cluster1, "cluster2": cluster2})
    return np.asarray(loss, dtype=np.float32)


# revision 5
# speedup vs baseline: 1.1514x; 1.1514x over previous
"""Trainium2 Bass kernel for nn_CustomLoss_45449343926664 (retrieval_knn).

loss = mse(mean(c1), mean(c2))
     + mean_i min_j ||c1_i - c2_j||^2
     + mean_k relu(0.1 - var(c1)_k)

Device computes the dominant term: per-row max_j(2<c1_i,c2_j> - |c2_j|^2)
(min-distance via d2 = |c1_i|^2 - that max). The tiny O(N*D) stats
(means / variances / |c1_i|^2) are host-side in fp64, fused into the
final scalar in _finish.

Sharding (8 cores = 4 i-groups x 2 j-halves): core c owns c1 rows
[2048*(c%4), 2048*(c%4+1)) and c2 rows [4096*(c//4), 4096*(c//4+1)).

Per core: 32 j-tiles of 128, each computed as two [128 j, 1024 i] PSUM
units (2 banks each, psum pool bufs=4 so the PE runs ahead of the
drains). Cross matmuls in "j-on-partitions" orientation (c2bT tile
stationary, c1bT moving, bf16, c1 pre-scaled by 2). The 64 units drain
through the only two engines with PSUM read ports, balanced to ~equal
busy time:

  - 17 units: DVE fused scalar_tensor_tensor drain
        zD' = max(psum + bias_j, zD)       (1 pass, per-i-half ping-pong)
  - 47 units: ACT activation(Identity, bias_j) -> bf16 z tiles; pairs
    of same-i-half z tiles fold via one DVE bf16 tensor_tensor max (2x
    mode, [128, 2048]) into per-half ping-pong accumulators.

Tail (per i-half, pipelined): max(accA halves) -> max(.., zD) -> 8 PE
transposes -> 3D reduce_max -> gmax[p, b] for query i = 128*b + p
(+ 2048*(c%4)). Host combines the two j-halves and finishes in fp64.
"""
import os
import sys

import numpy as np
import ml_dtypes

if os.path.isdir("/opt/trn_rl_repo") and "/opt/trn_rl_repo" not in sys.path:
    sys.path.insert(0, "/opt/trn_rl_repo")

from contextlib import ExitStack

import concourse.bass as bass
import concourse.tile as tile
from concourse import bacc, mybir
from concourse.bass_utils import run_bass_kernel_spmd
from concourse.masks import make_identity

F32 = mybir.dt.float32
BF16 = mybir.dt.bfloat16
BF16_NP = ml_dtypes.bfloat16
NEG_BIG = -3.0e38

N_CORES = 8
N1 = 8192            # cluster1 rows (total)
N2 = 8192            # cluster2 rows
D = 128              # feature dim = partition count
P = 128
I_GROUPS = 4
J_HALVES = 2
NI = N1 // I_GROUPS  # 2048 c1 rows per core
NJ = N2 // J_HALVES  # 4096 c2 rows per core
NJT = NJ // P        # 32 j-tiles of 128
MTI = NI // P        # 16 i-blocks of 128 (for the transpose tail)
FDI = 1024           # i-extent per PSUM unit (2 banks)
NU = NJT * 2         # 64 drain units (j-tile x i-half)
MM_SPLIT = 2         # matmuls per unit (one PSUM bank each)

# units drained directly by DVE (17 of 64); the rest go to ACT.
DVE_UNITS = frozenset(range(0, NU, 4)) | {17}
MIN_VARIANCE = 0.1

_cached = {}


def _build_program():
    """Build + compile the single-core SPMD program (same for all cores)."""
    nc = bacc.Bacc(
        "TRN2",
        target_bir_lowering=False,
        debug=False,
        enable_asserts=False,
        num_devices=N_CORES,
    )

    d_c1bT = nc.dram_tensor("c1bT", [D, NI], BF16, kind="ExternalInput").ap()
    d_c2bT = nc.dram_tensor("c2bT", [D, NJ], BF16, kind="ExternalInput").ap()
    d_sq2neg = nc.dram_tensor("sq2neg", [P, NJT], F32, kind="ExternalInput").ap()

    d_gmax = nc.dram_tensor("gmax", [P, MTI], F32, kind="ExternalOutput").ap()

    with tile.TileContext(nc) as tc, ExitStack() as ctx:
        const = ctx.enter_context(tc.tile_pool(name="const", bufs=1))
        zpool = [ctx.enter_context(tc.tile_pool(name=f"zp{h}", bufs=3))
                 for h in range(2)]
        psum = ctx.enter_context(tc.tile_pool(name="psum", bufs=4, space="PSUM"))

        t_c1bT = const.tile([P, NI], BF16)
        t_c2bT = const.tile([P, NJ], BF16)
        t_sq2neg = const.tile([P, NJT], F32)
        # per-i-half fold accumulators (ping-pong) + DVE-direct accumulators
        t_zA = [[const.tile([P, 2, FDI], BF16, name=f"zA{h}_{i}")
                 for i in range(2)] for h in range(2)]
        t_zD = [const.tile([P, NI], BF16, name=f"zD{i}") for i in range(2)]
        t_zfin = const.tile([P, NI], BF16)
        t_gmax = const.tile([P, MTI], F32)
        t_ident = const.tile([P, P], BF16)
        t_dummy = const.tile([P, 1], F32)

        # ---- input DMAs first (sync queue = otherwise idle) ----
        nc.sync.dma_start(t_sq2neg[:], d_sq2neg)
        nc.sync.dma_start(t_c2bT[:, 0 : 4 * P], d_c2bT[:, 0 : 4 * P])
        nc.sync.dma_start(t_c1bT[:, 0:FDI], d_c1bT[:, 0:FDI])
        nc.sync.dma_start(t_c1bT[:, FDI:], d_c1bT[:, FDI:])
        # remaining c2bT in 512-col chunks on gpsimd/scalar
        dma_engs = [nc.gpsimd, nc.scalar]
        for ci in range(7):
            lo, hi = 512 + ci * 512, min(1024 + ci * 512, NJ)
            dma_engs[ci % 2].dma_start(t_c2bT[:, lo:hi], d_c2bT[:, lo:hi])

        # warm the ACT function table (load ~1.3us) before the first drain
        nc.vector.memset(t_dummy[:], 1.0)
        nc.scalar.activation(t_dummy[:], t_dummy[:],
                             mybir.ActivationFunctionType.Identity, bias=0.0)

        # accumulator init on DVE (fast memset, DVE is idle pre-drain);
        # identity (transpose tail) on gpsimd
        nc.vector.memset(t_zD[0][:], NEG_BIG)
        nc.vector.memset(t_zA[0][0][:], NEG_BIG)
        nc.vector.memset(t_zA[1][0][:], NEG_BIG)
        make_identity(nc, t_ident[:])

        # ---- cross matmuls (j on partitions) + dual-engine drain ----
        nd = [0, 0]          # zD ping-pong index per i-half
        na = [0, 0]          # zA ping-pong index per i-half
        zpend = [None, None]  # partially-filled z pair per i-half
        for u in range(NU):
            t, h = u // 2, u % 2
            pt = psum.tile([P, FDI], F32, tag="pcross", name="pcross")
            lhsT = t_c2bT[:, t * P : (t + 1) * P]
            nmm = MM_SPLIT
            fd = FDI // nmm
            for c in range(nmm):
                nc.tensor.matmul(
                    pt[:, c * fd : (c + 1) * fd],
                    lhsT,
                    t_c1bT[:, h * FDI + c * fd : h * FDI + (c + 1) * fd],
                    start=True,
                    stop=True,
                )
            bias = t_sq2neg[:, t : t + 1]
            if u in DVE_UNITS:
                io = h * FDI
                nc.vector.scalar_tensor_tensor(
                    out=t_zD[(nd[h] + 1) % 2][:, io : io + FDI],
                    in0=pt[:],
                    scalar=bias,
                    in1=t_zD[nd[h] % 2][:, io : io + FDI],
                    op0=mybir.AluOpType.add,
                    op1=mybir.AluOpType.max,
                )
                nd[h] += 1
            else:
                if zpend[h] is None:
                    zpend[h] = (zpool[h].tile([P, 2, FDI], BF16, name=f"z{h}"), 0)
                zt, m = zpend[h]
                nc.scalar.activation(
                    zt[:, m], pt[:], mybir.ActivationFunctionType.Identity,
                    bias=bias, scale=1.0,
                )
                if m == 1:
                    # one bf16 2x tensor_max folds the pair into the two
                    # independent slots of this half's accumulator
                    nc.vector.tensor_max(t_zA[h][(na[h] + 1) % 2][:],
                                         t_zA[h][na[h] % 2][:], zt[:])
                    na[h] += 1
                    zpend[h] = None
                else:
                    zpend[h] = (zt, 1)
        for h in range(2):  # lone last ACT tile: pad pair slot with -inf
            if zpend[h] is not None:
                zt, _ = zpend[h]
                nc.vector.memset(zt[:, 1], NEG_BIG)
                nc.vector.tensor_max(t_zA[h][(na[h] + 1) % 2][:],
                                     t_zA[h][na[h] % 2][:], zt[:])
                na[h] += 1

        # ---- tail: per-half combine + partition-reduce via PE transpose ----
        for h in range(2):
            io = h * FDI
            acc = t_zA[h][na[h] % 2]
            nc.vector.tensor_max(t_zfin[:, io : io + FDI], acc[:, 0], acc[:, 1])
            nc.vector.tensor_max(t_zfin[:, io : io + FDI],
                                 t_zfin[:, io : io + FDI],
                                 t_zD[nd[h] % 2][:, io : io + FDI])
            ptr = psum.tile([P, MTI // 2, P], BF16, tag="pcross", name="ptr")
            for c in range(MTI // 2):
                nc.tensor.transpose(
                    ptr[:, c], t_zfin[:, io + c * P : io + (c + 1) * P],
                    t_ident[:])
            nc.vector.tensor_reduce(
                t_gmax[:, h * (MTI // 2) : (h + 1) * (MTI // 2)], ptr[:],
                axis=mybir.AxisListType.X, op=mybir.AluOpType.max)
        nc.sync.dma_start(d_gmax, t_gmax[:])

    nc.compile()
    return nc


def _prep_inputs(cluster1: np.ndarray, cluster2: np.ndarray):
    """Host-side sharding + operand layout prep."""
    c2b = cluster2.astype(BF16_NP)
    c2bT = np.ascontiguousarray(c2b.T)                       # [128, 8192] bf16
    sq2 = (c2b.astype(np.float32) ** 2).sum(axis=1)          # [8192] fp32
    sq2neg_h = []
    c2bT_h = []
    for h in range(J_HALVES):
        s = (-sq2[h * NJ : (h + 1) * NJ]).reshape(NJT, P).T
        sq2neg_h.append(np.ascontiguousarray(s).astype(np.float32))
        c2bT_h.append(np.ascontiguousarray(c2bT[:, h * NJ : (h + 1) * NJ]))

    c1bT_g = []
    for g in range(I_GROUPS):
        c1s = cluster1[g * NI : (g + 1) * NI]
        c1bT_g.append(np.ascontiguousarray((2.0 * c1s).astype(BF16_NP).T))

    in_maps = []
    for c in range(N_CORES):
        g, h = c % I_GROUPS, c // I_GROUPS
        in_maps.append({
            "c1bT": c1bT_g[g],
            "c2bT": c2bT_h[h],
            "sq2neg": sq2neg_h[h],
        })
    return in_maps


def _finish(results, cluster1, cluster2) -> np.float32:
    """Combine per-core partials + host-side O(N*D) stats (fp64)."""
    c1 = np.asarray(cluster1, np.float64)
    c2 = np.asarray(cluster2, np.float64)

    # distance term: d2_i = |c1_i|^2 - max_j(2<c1,c2> - |c2_j|^2)
    # (cross/bias computed on device from bf16-rounded operands)
    sq1 = (c1 * c1).sum(axis=1)  # [8192]
    dist_sum = 0.0
    for g in range(I_GROUPS):
        gm0 = np.asarray(results[g]["gmax"], np.float64)            # [128, 16]
        gm1 = np.asarray(results[g + I_GROUPS]["gmax"], np.float64)
        # column b covers i-half b//8, block b%8: i_local = 1024*(b//8)
        #   + 128*(b%8) + p  == 128*b + p  (b ordered h-major == block-major)
        gm = np.maximum(gm0, gm1)                  # [p, b]
        gmax_rows = gm.T.reshape(NI)               # [2048] in i_local order
        dist_sum += (sq1[g * NI : (g + 1) * NI] - gmax_rows).sum()
    dist = dist_sum / N1

    m1 = c1.mean(axis=0)
    m2 = c2.mean(axis=0)
    mean_loss = ((m1 - m2) ** 2).mean()
    var = (c1 * c1).mean(axis=0) - m1 ** 2
    disp = np.maximum(MIN_VARIANCE - var, 0.0).mean()
    return np.float32(mean_loss + dist + disp)


def _run(inputs, trace=False, **kwargs):
    """Run on the 8 NeuronCores. Returns (loss_scalar, BassKernelResults)."""
    if "nc" not in _cached:
        _cached["nc"] = _build_program()
    nc = _cached["nc"]
    c1 = np.asarray(inputs["cluster1"], np.float32)
    c2 = np.asarray(inputs["cluster2"], np.float32)
    in_maps = _prep_inputs(c1, c2)
    res = run_bass_kernel_spmd(nc, in_maps, list(range(N_CORES)), trace=trace,
                               **kwargs)
    loss = _finish(res.results, c1, c2)
    return loss, res


def kernel(cluster1: np.ndarray, cluster2: np.ndarray) -> np.ndarray:
    loss, _ = _run({"cluster1": cluster1, "cluster2": cluster2})
    return np.asarray(loss, dtype=np.float32)
